# revision 5
# baseline (speedup 1.0000x reference)
"""Trainium2 Bass kernel for CompressDCT encoder (per-8x8-block 2D DCT,
quantize, round-nearest-even, clamp).

Approach: single-pass kron DCT
------------------------------
For an 8x8 block X, the 2D DCT is Y = D X D^T, i.e. in vectorized form
    y_vec = (D (x) D) x_vec,          (x) = Kronecker product, 64x64.
Two 8x8 blocks are packed per 128-partition column, so ONE stationary
matrix  KRON2 = blockdiag(D(x)D, D(x)D)  [128x128] performs the whole
2D DCT for two blocks per moving column in a single matmul pass:

    out[o, f] = sum_p KRON2[o, p] * x[p, f]

with p = 64*parity + (r*8+c)  (input sample index inside the block pair)
and  o = 64*parity + (k*8+l)  (DCT coefficient index).   The host packs
x into this [128, 32768] layout (32768 block-pairs per core) and unpacks
the result; both are cheap numpy reshapes/transposes off-device.

Per core: 64 matmuls (N=512, fp16 -> 1 cycle/row, single stationary for
the whole kernel), then PSUM fp32 -> SBUF int8 saturating copies
(alternating ScalarE/VectorE; int8 conversion is exactly
round-nearest-even + clamp to [-128,127], matching bit=8), then
contiguous-line DMAs (4KB descriptors in, 2KB out).

The quantization table is folded EXACTLY into the stationary matrix
(rows of KRON2 scaled by 1/q[k,l]) - no rank-1 requirement.

Numerics: fp16 inputs + fp16 stationary + fp32 PSUM accumulation flips
~0.02% of outputs by exactly 1 quantization step vs the fp32 reference
(CPU sim: 6594/33.5M, L2 rel 0.0135 vs gate 2e-2; baseline f32r 2-pass
was 5344 flips / 0.0121).

Sharding: pure data-parallel, images [256*i : 256*(i+1)] per core.
"""
import os
import sys

import numpy as np

for _p in ("/opt/trn_rl_repo", "/root/.axon_site/_ro/trn_rl_repo"):
    if _p not in sys.path:
        sys.path.append(_p)

# The bass execute path runs through the axon PJRT proxy; make sure a
# JAX_PLATFORMS override (e.g. cpu-pinned reference runners) doesn't hide
# the axon backend, as long as jax hasn't been imported yet.
if os.environ.get("AXON_H4_ENABLED") == "1" and "jax" not in sys.modules:
    if "axon" not in os.environ.get("JAX_PLATFORMS", "axon"):
        os.environ["JAX_PLATFORMS"] = "axon"

import concourse.bacc as bacc
import concourse.tile as tile
from concourse import mybir
from concourse.bass_utils import run_bass_kernel_spmd

F16 = mybir.dt.float16
F32 = mybir.dt.float32
I8 = mybir.dt.int8

N_CORES = 8
P = 128                      # partitions = 2 blocks x 64 samples
IMGS_PER_CORE = 256          # (32/8) * 64 images of 128x128
COLS = IMGS_PER_CORE * 256 // 2   # block-pairs per core = 32768
DMA_CHUNK = 2048             # columns per input/output DMA
COPY_CHUNK = 1024            # columns per PSUM->SBUF copy
MM_CHUNK = 512               # columns per matmul (one PSUM bank fp32)
MAGIC = 12582912.0           # 1.5 * 2**23, fp32 RNE-to-integer trick

LAST_RESULTS = None


def _dct_matrix() -> np.ndarray:
    n = np.arange(8)
    k = np.arange(8)[:, None]
    D = np.cos(np.pi * (2 * n + 1) * k / 16.0)
    D *= np.sqrt(2.0 / 8.0)
    D[0] *= 1.0 / np.sqrt(2.0)
    return D.astype(np.float64)


def _build(bit: int, q_table: np.ndarray):
    """Build + compile the per-core Bass program."""
    int8_out = bit == 8

    D = _dct_matrix()
    KR = np.kron(D, D)                       # [64 (k,l), 64 (r,c)]
    KR = KR / q_table.astype(np.float64).reshape(64)[:, None]
    KR2 = np.kron(np.eye(2), KR)             # [128, 128] block-diagonal
    kmat = np.ascontiguousarray(KR2.T).astype(np.float16)  # lhsT [p, o]

    nc = bacc.Bacc("TRN2", target_bir_lowering=False, debug=False)
    x_ap = nc.dram_tensor("x", [P, COLS], F16, kind="ExternalInput").ap()
    out_dt = I8 if int8_out else F32
    out_ap = nc.dram_tensor("out", [P, COLS], out_dt, kind="ExternalOutput").ap()
    KMAT = nc.inline_tensor(kmat, name="kmat")

    lo = float(-(2.0 ** (bit - 1)))
    hi = float(2.0 ** (bit - 1) - 1)

    n_dma = COLS // DMA_CHUNK                # 16
    n_copy = DMA_CHUNK // COPY_CHUNK         # 2 per DMA chunk
    n_mm = COPY_CHUNK // MM_CHUNK            # 2 per copy chunk

    with tile.TileContext(nc) as tc:
        with tc.tile_pool(name="const", bufs=1) as cpool, \
             tc.tile_pool(name="inp", bufs=3) as ipool, \
             tc.tile_pool(name="outp", bufs=3) as opool, \
             tc.tile_pool(name="mid", bufs=2) as mpool, \
             tc.tile_pool(name="ps", bufs=3, space="PSUM") as pspool:

            kt = cpool.tile([P, P], F16, tag="kt")
            nc.sync.dma_start(out=kt[:], in_=KMAT.ap())

            ci = 0                           # global copy index
            for c in range(n_dma):
                xin = ipool.tile([P, DMA_CHUNK], F16, tag="xin")
                # scalar/sync have hardware DGE queues; gpsimd's is software
                # (~1us per issue) and throttles the input stream ramp.
                nc.scalar.dma_start(
                    out=xin[:],
                    in_=x_ap[:, c * DMA_CHUNK:(c + 1) * DMA_CHUNK],
                )
                oc = opool.tile([P, DMA_CHUNK], out_dt, tag="oc")

                for k in range(n_copy):
                    ps = pspool.tile([P, COPY_CHUNK], F32, tag="ps")
                    for m in range(n_mm):
                        xo = k * COPY_CHUNK + m * MM_CHUNK
                        nc.tensor.matmul(
                            ps[:, m * MM_CHUNK:(m + 1) * MM_CHUNK],
                            kt[:],
                            xin[:, xo:xo + MM_CHUNK],
                            start=True,
                            stop=True,
                        )
                    dst = oc[:, k * COPY_CHUNK:(k + 1) * COPY_CHUNK]
                    if int8_out:
                        # fp32 -> int8 == RNE + saturate to [-128,127]
                        if ci % 2 == 0:
                            nc.scalar.activation(
                                dst, ps[:], mybir.ActivationFunctionType.Copy
                            )
                        else:
                            nc.vector.tensor_copy(dst, ps[:])
                    else:
                        rnd = mpool.tile([P, COPY_CHUNK], F32, tag="rnd")
                        nc.vector.tensor_scalar(
                            rnd[:], ps[:], MAGIC, MAGIC,
                            mybir.AluOpType.add, mybir.AluOpType.subtract,
                        )
                        nc.vector.tensor_scalar(
                            dst, rnd[:], hi, lo,
                            mybir.AluOpType.min, mybir.AluOpType.max,
                        )
                    ci += 1

                nc.sync.dma_start(
                    out=out_ap[:, c * DMA_CHUNK:(c + 1) * DMA_CHUNK],
                    in_=oc[:],
                )

    nc.compile()
    return nc


_CACHE = {}


def _get_program(bit: int, q_table: np.ndarray):
    key = (bit, q_table.tobytes())
    if key not in _CACHE:
        _CACHE[key] = _build(bit, q_table)
    return _CACHE[key]


def _pack(x: np.ndarray) -> np.ndarray:
    """(N, C, 128, 128) fp32 -> [cores, 128, COLS] fp16 device layout."""
    N, C, H, W = x.shape
    x16 = x.astype(np.float16)
    # -> (cores, img, bh, r, bw, c) -> (cores, img, bh, bw, r, c)
    b = x16.reshape(N_CORES, IMGS_PER_CORE, 16, 8, 16, 8)
    b = b.transpose(0, 1, 2, 4, 3, 5).reshape(N_CORES, -1, 64)  # [cores, B, 64]
    # block index b = f*2 + parity ; partition = parity*64 + (r*8+c)
    b = b.reshape(N_CORES, COLS, 2, 64).transpose(0, 2, 3, 1)   # [cores, 2, 64, COLS]
    return np.ascontiguousarray(b.reshape(N_CORES, P, COLS))


def _unpack(dev: np.ndarray) -> np.ndarray:
    """[cores, 128, COLS] -> (cores*IMGS, 128, 128) float32."""
    d = dev.reshape(N_CORES, 2, 64, COLS).transpose(0, 3, 1, 2)  # [cores, COLS, 2, 64]
    d = d.reshape(N_CORES, IMGS_PER_CORE, 16, 16, 8, 8)          # (img, bh, bw, k, l)
    d = d.transpose(0, 1, 2, 4, 3, 5)                            # (img, bh, k, bw, l)
    return d.reshape(N_CORES * IMGS_PER_CORE, 128, 128).astype(np.float32)


def kernel(x, q_table, bit):
    x = np.asarray(x, dtype=np.float32)
    q_table = np.asarray(q_table, dtype=np.float32)
    bit = int(bit)
    N, C, H, W = x.shape
    assert (H, W) == (128, 128) and N * C == N_CORES * IMGS_PER_CORE, (
        "kernel hardcoded for (32,64,128,128)"
    )

    nc = _get_program(bit, q_table)

    packed = _pack(x)
    in_maps = [{"x": packed[i]} for i in range(N_CORES)]

    res = run_bass_kernel_spmd(nc, in_maps, core_ids=list(range(N_CORES)))
    global LAST_RESULTS
    LAST_RESULTS = res

    dev = np.stack([res.results[i]["out"] for i in range(N_CORES)])
    out = _unpack(dev).reshape(N, C, H, W)
    return out
